# revision 58
# baseline (speedup 1.0000x reference)
"""Tensor-parallel Llama MHA kernel for 8 TRN2 NeuronCores.

Problem: B=2, S=2048, HIDDEN=2048, 16 heads x head_dim 128, fp32, RoPE + causal.

Sharding: 8 cores = 2 (batch) x 4 (head groups of 4 heads).  Each core computes
q/k/v projections for its 4 heads, flash-style causal attention, and a partial
o_proj (attn_out_heads @ Wo[:, heads].T).  The full output is the sum of the 4
head-group partials per batch element, done on the host after gather (partials
are shipped bf16, summed fp32).

Per-core design (PE streaming roofline ~280us; everything else hides under it):
  - All matmul operands bf16 (1 col/cycle PE rate), fp32 PSUM accumulation.
  - Input DMAs: x is host-swizzled seq-chunk-major so the first projection
    matmul is gated on ~1MB, not the whole 8.4MB; weight/x/constant loads are
    spread across the scalar/sync/vector DGE queues so issue overhead
    parallelizes.
  - RoPE entirely off the PE: rotate_half is an SBUF->SBUF DMA partition
    swap (DVE lanes cannot cross partitions; DMA can) with the sign folded
    into the sin table, then three bf16 DVE multiplies per head tile,
    emitted one seq chunk late so nothing ever waits on the eviction.
  - qc=0's S matmuls + exps are pre-staged into the projection tail (the
    v-projection provides PE cover) so the filler-less first attention
    phase starts with ready-to-consume pairs.
  - Attention (per head, per 512-query chunk): S^T blocks [k=128, q<=512] with
    causally-trimmed widths on diagonal chunks (bank-aligned packing in PSUM),
    exp on ScalarE, triangle masks as [128,128] DVE multiplies, softmax
    denominator accumulated on DVE in bf16 and reduced with a single
    ones-matmul per (head, chunk); 1/l = exp(-ln(l)) on ScalarE (one ACT
    table set covers Exp+Ln+Copy).
  - o_proj matmuls are interleaved into the attention pair pipeline as PE
    filler (one 4-matmul tile per consumed pair) so the PE never waits for
    ScalarE exp; a few tiles are held in reserve to cover the final
    normalization chain.
  - Output copied PSUM->SBUF on alternating ScalarE/DVE and DMA'd out bf16.
  - Post pass: TRN2 instructions carry at most one sync wait; excess waits
    are peeled onto same-engine event-semaphore instructions.
"""

import math
from collections import deque

import numpy as np

HIDDEN = 2048
NUM_HEADS = 16
HEAD_DIM = 128
BATCH = 2
SEQ = 2048
ROPE_BASE = 10000.0

N_CORES = 8
N_HGROUPS = N_CORES // BATCH          # 4 head-groups
H_LOCAL = NUM_HEADS // N_HGROUPS      # 4 heads per core
D = HEAD_DIM                          # 128
SQ = 512                              # query chunk
KB = 128                              # key block


def build_bass(seq=SEQ, hid=HIDDEN, h_local=H_LOCAL):
    """Build the single-core Bass program (SPMD: same program on all cores)."""
    import concourse.bass as bass
    import concourse.tile as tile
    from concourse import mybir

    f32 = mybir.dt.float32
    bf16 = mybir.dt.bfloat16
    EXP = mybir.ActivationFunctionType.Exp
    LN = mybir.ActivationFunctionType.Ln

    n_qc = seq // SQ                  # query chunks (4)
    n_kc = hid // 128                 # hidden (contraction) chunks (16)
    n_sc = seq // SQ                  # seq chunks (4)
    n_ms = SQ // 128                  # 128-row subchunks in a 512 chunk
    n_on = hid // SQ                  # output col chunks of 512
    M = h_local * D                   # projection output width (512)
    isqrt_d = 1.0 / math.sqrt(D)

    nc = bass.Bass(target_bir_lowering=False, trn_type="TRN2")

    # ---- DRAM I/O (host pre-swizzled, bf16) ----
    # x: [128, n(seq chunk), kc, 512] flattened — seq-chunk-major
    xS = nc.dram_tensor("xS", [128, n_sc * n_kc * SQ], bf16, kind="ExternalInput")
    wqS = nc.dram_tensor("wqS", [128, n_kc * M], bf16, kind="ExternalInput")
    wkS = nc.dram_tensor("wkS", [128, n_kc * M], bf16, kind="ExternalInput")
    wvS = nc.dram_tensor("wvS", [128, n_kc * M], bf16, kind="ExternalInput")
    woS = nc.dram_tensor("woS", [128, h_local * hid], bf16, kind="ExternalInput")
    cosT = nc.dram_tensor("cosT", [D, seq], bf16, kind="ExternalInput")
    # sin with rows 0..63 negated: rotate_half's sign, folded host-side
    sinS = nc.dram_tensor("sinS", [D, seq], bf16, kind="ExternalInput")
    triS = nc.dram_tensor("triS", [KB, KB], bf16, kind="ExternalInput")
    onesd = nc.dram_tensor("ones", [128, 128], bf16, kind="ExternalInput")
    out = nc.dram_tensor("out", [seq, hid], bf16, kind="ExternalOutput")

    with tile.TileContext(nc) as tc:
        with tc.tile_pool(name="persist", bufs=1) as persist:
            q_sb = persist.tile([128, h_local, seq], bf16)    # [d, head, seq]
            k_sb = persist.tile([128, h_local, seq], bf16)
            v_sb = persist.tile([128, seq // 128, M], bf16)   # [s%128, schunk, h*d]
            wo_sb = persist.tile([128, h_local, hid], bf16)   # [d, head, hid]
            cos_sb = persist.tile([128, seq], bf16)
            sin_sb = persist.tile([128, seq], bf16)
            ones_sb = persist.tile([128, 128], bf16)
            tri_sb = persist.tile([KB, KB], bf16)

            # ================= Phase 1: projections + RoPE =================
            with (
                tc.tile_pool(name="proj", bufs=1) as proj,
                tc.tile_pool(name="psum_p", bufs=1, space="PSUM") as psum_p,
            ):
                x_res = proj.tile([128, n_sc, n_kc, SQ], bf16)

                def load_w_groups(w_dram, queue):
                    groups = []
                    for g in range(8):
                        wt = proj.tile([128, 2, M], bf16, tag="w", bufs=16)
                        queue.dma_start(
                            out=wt,
                            in_=w_dram[:, g * 2 * M : (g + 1) * 2 * M].rearrange(
                                "p (kc m) -> p kc m", m=M
                            ),
                        )
                        groups.append(wt)
                    return groups

                # scalar DGE queue: q then k weights (scalar engine is idle
                # until the first eviction ~15us in)
                wq_g = load_w_groups(wqS, nc.scalar)
                wk_g = load_w_groups(wkS, nc.scalar)

                # sync DGE queue, in need-order: x chunk 0 in fine kc-pair
                # slices (first matmul gated on wq g0 + first 256KB), chunk 1
                # in quarters, then constants, then x chunks 2-3, v weights,
                # Wo
                def load_x_chunk(n, step=4):
                    for g0 in range(0, n_kc, step):
                        c0 = (n * n_kc + g0) * SQ
                        nc.sync.dma_start(
                            out=x_res[:, n, g0 : g0 + step, :],
                            in_=xS[:, c0 : c0 + step * SQ].rearrange(
                                "p (kc s) -> p kc s", s=SQ
                            ),
                        )

                load_x_chunk(0, step=2)
                load_x_chunk(1)
                nc.sync.dma_start(out=cos_sb, in_=cosT[:])
                nc.sync.dma_start(out=sin_sb, in_=sinS[:])
                nc.sync.dma_start(out=tri_sb, in_=triS[:])
                nc.sync.dma_start(out=ones_sb, in_=onesd[:])
                load_x_chunk(2)
                load_x_chunk(3)
                wv_g = load_w_groups(wvS, nc.sync)
                nc.sync.dma_start(
                    out=wo_sb,
                    in_=woS[:].rearrange("p (h n) -> p h n", n=hid),
                )

                def evac_chunk(ps, n):
                    """Evacuate chunk n's projection PSUM to bf16 SBUF and
                    kick off the rotate_half partition-swap DMAs (SBUF->SBUF;
                    DVE lanes cannot cross partitions, DMA can).  Emitted
                    inline (no PE instructions) so the PSUM slots' readers
                    exist before slot reuse."""
                    qraws, qrots = [], []
                    for t in range(2):
                        qraw = proj.tile([128, 1024], bf16, tag="qraw", bufs=4)
                        if t == 0:
                            nc.scalar.copy(out=qraw, in_=ps[t])
                        else:
                            nc.vector.tensor_copy(qraw, ps[t])
                        qrot = proj.tile([128, 1024], bf16, tag="qrot", bufs=4)
                        nc.scalar.dma_start(out=qrot[0:64, :], in_=qraw[64:128, :])
                        nc.scalar.dma_start(out=qrot[64:128, :], in_=qraw[0:64, :])
                        qraws.append(qraw)
                        qrots.append(qrot)
                    return qraws, qrots

                def rope_chunk(st):
                    """RoPE on DVE only: dst = qraw*cos + swap(qraw)*sinS
                    (sign folded into sinS).  Called one seq chunk late so
                    the swap DMAs are long done."""
                    (qraws, qrots, dst, n) = st
                    cseg = cos_sb[:, n * SQ : (n + 1) * SQ]
                    sseg = sin_sb[:, n * SQ : (n + 1) * SQ]
                    for t in range(2):
                        for p in range(2):
                            m = 2 * t + p
                            qt = qraws[t][:, p * SQ : (p + 1) * SQ]
                            dstv = dst[:, m, n * SQ : (n + 1) * SQ]
                            tsin = proj.tile([128, SQ], bf16, tag="tsin", bufs=3)
                            nc.vector.tensor_mul(
                                tsin, qrots[t][:, p * SQ : (p + 1) * SQ], sseg
                            )
                            nc.vector.tensor_mul(dstv, qt, cseg)
                            nc.vector.tensor_add(dstv, dstv, tsin)

                prestaged = {}

                def emit_prestage(h, kbp):
                    """Pre-stage one qc=0 S-pair + exp under v-proj PE cover
                    (the filler-less first attention phase otherwise runs a
                    latency-bound S->exp->mask->AV lockstep)."""
                    kb0 = 2 * kbp
                    w0, w1 = 512 - 128 * kb0, 512 - 128 * (kb0 + 1)
                    qo0, qo1 = 512 - w0, 512 - w1
                    s_pre = psum_p.tile(
                        [128, 1024], f32, tag="ps", bufs=4,
                        name=f"spre_{h}_{kbp}",
                    )
                    nc.tensor.matmul(
                        s_pre[:, 0:w0],
                        lhsT=k_sb[:, h, kb0 * KB : (kb0 + 1) * KB],
                        rhs=q_sb[:, h, qo0:SQ],
                        start=True,
                        stop=True,
                        skip_group_check=True,
                    )
                    nc.tensor.matmul(
                        s_pre[:, 512 : 512 + w1],
                        lhsT=k_sb[:, h, (kb0 + 1) * KB : (kb0 + 2) * KB],
                        rhs=q_sb[:, h, qo1:SQ],
                        start=True,
                        stop=True,
                        skip_group_check=True,
                    )
                    p_pre = persist.tile([128, 1024], bf16, name=f"ppre_{h}_{kbp}")
                    if w0 + w1 <= 640:
                        nc.scalar.activation(
                            p_pre[:, 0:w0], s_pre[:, 0:w0], EXP, scale=isqrt_d
                        )
                        nc.scalar.activation(
                            p_pre[:, 512 : 512 + w1],
                            s_pre[:, 512 : 512 + w1],
                            EXP,
                            scale=isqrt_d,
                        )
                    else:
                        nc.scalar.activation(p_pre, s_pre, EXP, scale=isqrt_d)
                    prestaged[(h, kbp)] = p_pre

                PRE = [(h, kbp) for h in range(h_local) for kbp in range(2)]
                pre_sched = {1: PRE[0:3], 2: PRE[3:6], 3: PRE[6:8]}

                pend_rope = None
                for proj_i, (w_g, dst, is_v) in enumerate(
                    ((wq_g, q_sb, False), (wk_g, k_sb, False), (wv_g, v_sb, True))
                ):
                    for n in range(n_sc):
                        ps = [
                            psum_p.tile(
                                [128, 1024], f32, tag="ps", bufs=4,
                                name=f"ps_{proj_i}_{n}_{t}",
                            )
                            for t in range(2)
                        ]

                        def acc_slice(i):
                            return ps[i // 2][:, (i % 2) * SQ : (i % 2) * SQ + SQ]

                        for kc in range(n_kc):
                            g, gi = kc // 2, kc % 2
                            start = kc == 0
                            stop = kc == n_kc - 1
                            if not is_v:
                                x_t = x_res[:, n, kc, :]
                                for m in range(h_local):
                                    nc.tensor.matmul(
                                        acc_slice(m),
                                        lhsT=w_g[g][:, gi, m * D : (m + 1) * D],
                                        rhs=x_t,
                                        start=start,
                                        stop=stop,
                                    )
                            else:
                                for sub in range(n_ms):
                                    nc.tensor.matmul(
                                        acc_slice(sub),
                                        lhsT=x_res[
                                            :, n, kc, sub * 128 : (sub + 1) * 128
                                        ],
                                        rhs=w_g[g][:, gi, :],
                                        start=start,
                                        stop=stop,
                                    )
                        if is_v:
                            # all v evacuations on DVE: keep ScalarE clear
                            # for the interleaved pre-staged exps below
                            for sub in range(n_ms):
                                nc.vector.tensor_copy(
                                    v_sb[:, n * n_ms + sub, :],
                                    acc_slice(sub),
                                )
                            if pend_rope is not None:
                                rope_chunk(pend_rope)
                                pend_rope = None
                            for h, kbp in pre_sched.get(n, ()):
                                emit_prestage(h, kbp)
                        else:
                            qraws, qrots = evac_chunk(ps, n)
                            if pend_rope is not None:
                                rope_chunk(pend_rope)
                            pend_rope = (qraws, qrots, dst, n)
                if pend_rope is not None:
                    rope_chunk(pend_rope)
                    pend_rope = None

            # ================= Phase 2: attention + o_proj =================
            with (
                tc.tile_pool(name="attn", bufs=1) as attn,
                tc.tile_pool(name="psum_a", bufs=1, space="PSUM") as psum_a,
            ):
                pend_ot = {}
                opq = deque()       # pending o_proj (qc, ms, on) tiles
                evac_ctr = [0]

                def emit_oproj_tile(drain=False):
                    qc, ms, on = opq.popleft()
                    oo = psum_a.tile([128, SQ], f32, tag="oo", bufs=2)
                    for h in range(h_local):
                        nc.tensor.matmul(
                            oo,
                            lhsT=pend_ot[(qc, h)][:, ms * 128 : (ms + 1) * 128],
                            rhs=wo_sb[:, h, on * SQ : (on + 1) * SQ],
                            start=(h == 0),
                            stop=(h == h_local - 1),
                            skip_group_check=True,
                        )
                    out_t = attn.tile([128, SQ], bf16, tag="outt", bufs=8)
                    # during attention ScalarE is saturated by exp + the 1/l
                    # chain, so evacuate on DVE; in the final drain (no more
                    # exps) alternate both engines
                    evac_ctr[0] += 1
                    if drain and evac_ctr[0] % 2 == 0:
                        nc.scalar.copy(out=out_t, in_=oo)
                    else:
                        nc.vector.tensor_copy(out_t, oo)
                    nc.sync.dma_start(
                        out=out[
                            qc * SQ + ms * 128 : qc * SQ + (ms + 1) * 128,
                            on * SQ : (on + 1) * SQ,
                        ],
                        in_=out_t,
                    )

                def consume(st):
                    """Mask, l-accumulate, and AV the pair LAG steps behind."""
                    (kb0, p_sb, unit, first, widths) = st
                    (w0, qo0, w1, qo1, diag) = widths
                    (h, qc, n_kb) = unit["h"], unit["qc"], unit["n_kb"]
                    if first:
                        # lazy: allocate at first use so the previous
                        # generation's readers are already emitted
                        unit["o_ps"] = psum_a.tile(
                            [128, SQ], f32, tag="o", bufs=2,
                            name=f"ops_{qc}_{h}",
                        )
                        unit["l_acc"] = attn.tile(
                            [128, SQ], bf16, tag="lacc", bufs=2,
                            name=f"lacc_{qc}_{h}",
                        )
                    o_ps, l_acc = unit["o_ps"], unit["l_acc"]
                    if diag:
                        # triangle masks: block b's first 128 q-columns
                        nc.vector.tensor_mul(
                            p_sb[:, 0:128], p_sb[:, 0:128], tri_sb
                        )
                        nc.vector.tensor_mul(
                            p_sb[:, 512:640], p_sb[:, 512:640], tri_sb
                        )
                    # AV matmuls
                    for b, (w, qo) in enumerate(((w0, qo0), (w1, qo1))):
                        kb = kb0 + b
                        nc.tensor.matmul(
                            o_ps[:, qo:SQ],
                            lhsT=v_sb[:, kb, h * D : (h + 1) * D],
                            rhs=p_sb[:, b * 512 : b * 512 + w],
                            start=(kb == 0),
                            stop=(kb == n_kb - 1),
                            skip_group_check=True,
                        )
                    # softmax denominator accumulation on DVE (bf16)
                    if first:
                        if diag:
                            nc.vector.tensor_copy(
                                l_acc[:, qo0:SQ], p_sb[:, 0:w0]
                            )
                        else:
                            nc.vector.tensor_add(
                                l_acc, p_sb[:, 0:512], p_sb[:, 512:1024]
                            )
                            return
                    else:
                        nc.vector.tensor_add(
                            l_acc[:, qo0:SQ], l_acc[:, qo0:SQ], p_sb[:, 0:w0]
                        )
                    nc.vector.tensor_add(
                        l_acc[:, qo1:SQ],
                        l_acc[:, qo1:SQ],
                        p_sb[:, 512 : 512 + w1],
                    )

                def unit_end(unit):
                    (o_ps, l_acc) = unit["o_ps"], unit["l_acc"]
                    (h, qc) = unit["h"], unit["qc"]
                    l_ps = psum_a.tile([128, SQ], f32, tag="oo", bufs=2)
                    nc.tensor.matmul(
                        l_ps,
                        lhsT=ones_sb,
                        rhs=l_acc,
                        start=True,
                        stop=True,
                        skip_group_check=True,
                    )
                    # 1/l = exp(-ln(l)) on ScalarE (l > 0 always)
                    lg = attn.tile([128, SQ], f32, tag="lg", bufs=2)
                    nc.scalar.activation(lg, l_ps, LN)
                    linv = attn.tile([128, SQ], f32, tag="linv", bufs=2)
                    nc.scalar.activation(linv, lg, EXP, scale=-1.0)
                    ot = attn.tile([128, SQ], bf16, tag="ot", bufs=12)
                    nc.vector.tensor_mul(ot, o_ps, linv)
                    pend_ot[(qc, h)] = ot
                    if h == h_local - 1:
                        for ms in range(n_ms):
                            for on in range(n_on):
                                opq.append((qc, ms, on))

                LAG = 3          # pairs in flight before consumption: the
                                 # S->exp->mask->AV chain is ~2us while an
                                 # S-pair is only ~0.9us of PE work, so two
                                 # pairs of lookahead are needed to hide it
                fifo = deque()   # pending pair tasks
                uend_q = deque() # units whose last pair was consumed
                state = {"pops": [], "ci": 0}

                def step_consume():
                    st = fifo.popleft()
                    consume(st)
                    state["ci"] += 1
                    ci = state["ci"]
                    if ci < len(state["pops"]):
                        for _ in range(state["pops"][ci]):
                            if opq:
                                emit_oproj_tile()
                    if st[0] + 2 == st[2]["n_kb"]:  # last pair of its unit
                        uend_q.append(st[2])

                # Phase order: qc=0 first (the filler-less phase runs in
                # latency-bound lockstep, so keep it as short as possible —
                # qc=0 is only 8 pairs), then qc=1 paced by o_proj(0), qc=2
                # by o_proj(1), qc=3 by o_proj(2); the rest (plus a 6-tile
                # reserve covering the final normalize chain and the drain's
                # PSUM-rotation warmup) drain at the end.
                phases = [
                    ([(0, 0), (0, 1), (0, 2), (0, 3)], 0),
                    ([(1, 0), (1, 1), (1, 2), (1, 3)], 10),
                    ([(2, 0), (2, 1), (2, 2), (2, 3)], 16),
                    ([(3, 0), (3, 1), (3, 2), (3, 3)], 16),
                ]
                for units, avail in phases:
                    n_consume = sum(2 * (qc + 1) for qc, _ in units)
                    pops = [0] * (n_consume + 1)
                    for i in range(avail):
                        pops[
                            min(n_consume, int(round((i + 1) * n_consume / (avail + 1))))
                        ] += 1
                    state["pops"] = pops
                    state["ci"] = 0
                    for qc, h in units:
                        n_kb = (qc + 1) * (SQ // KB)
                        unit = {
                            "h": h, "qc": qc, "n_kb": n_kb,
                            "o_ps": None, "l_acc": None,
                        }
                        for kbp in range(n_kb // 2):
                            kb0 = 2 * kbp
                            j0 = kb0 - (SQ // KB) * qc
                            diag = j0 >= 0
                            if diag:
                                w0, w1 = 512 - 128 * j0, 512 - 128 * (j0 + 1)
                            else:
                                w0, w1 = 512, 512
                            qo0, qo1 = 512 - w0, 512 - w1
                            if qc == 0 and (h, kbp) in prestaged:
                                # S + exp already emitted in the proj tail
                                while uend_q:
                                    unit_end(uend_q.popleft())
                                fifo.append((
                                    kb0, prestaged[(h, kbp)], unit,
                                    kbp == 0, (w0, qo0, w1, qo1, diag),
                                ))
                                if len(fifo) > LAG:
                                    step_consume()
                                continue
                            s_ps = psum_a.tile([128, 1024], f32, tag="s", bufs=2)
                            q0 = q_sb[:, h, qc * SQ + qo0 : (qc + 1) * SQ]
                            q1 = q_sb[:, h, qc * SQ + qo1 : (qc + 1) * SQ]
                            nc.tensor.matmul(
                                s_ps[:, 0:w0],
                                lhsT=k_sb[:, h, kb0 * KB : (kb0 + 1) * KB],
                                rhs=q0,
                                start=True,
                                stop=True,
                                skip_group_check=True,
                            )
                            nc.tensor.matmul(
                                s_ps[:, 512 : 512 + w1],
                                lhsT=k_sb[:, h, (kb0 + 1) * KB : (kb0 + 2) * KB],
                                rhs=q1,
                                start=True,
                                stop=True,
                                skip_group_check=True,
                            )
                            p_sb = attn.tile([128, 1024], bf16, tag="p", bufs=6)
                            if diag and w0 + w1 <= 640:
                                # narrow diagonal pair: two small ACTs win
                                nc.scalar.activation(
                                    p_sb[:, 0:w0], s_ps[:, 0:w0], EXP,
                                    scale=isqrt_d,
                                )
                                nc.scalar.activation(
                                    p_sb[:, 512 : 512 + w1],
                                    s_ps[:, 512 : 512 + w1],
                                    EXP,
                                    scale=isqrt_d,
                                )
                            else:
                                # full-width ACT; exp of the stale gap columns
                                # is never read downstream
                                nc.scalar.activation(
                                    p_sb, s_ps, EXP, scale=isqrt_d
                                )
                            while uend_q:
                                unit_end(uend_q.popleft())
                            fifo.append((
                                kb0, p_sb, unit,
                                kbp == 0, (w0, qo0, w1, qo1, diag),
                            ))
                            if len(fifo) > LAG:
                                step_consume()
                # flush: remaining pairs, unit ends, o_proj tiles
                while fifo:
                    step_consume()
                while uend_q:
                    unit_end(uend_q.popleft())
                while opq:
                    emit_oproj_tile(drain=True)

    # Finalize (assigns semaphore waits), then legalize: TRN2 instructions
    # accept only ONE sync wait each (EventSemaphore: two, InstISA: zero).
    nc.to_json_bytes()
    _legalize_waits(nc, mybir)
    return nc


def _legalize_waits(nc, mybir):
    """TRN2 instructions carry at most ONE sync wait (InstEventSemaphore:
    two; raw InstISA: none).  Peel excess waits onto event-semaphore
    instructions inserted immediately before, on the same engine sequencer
    (program order keeps the semantics)."""
    nfix = 0
    for f in nc.m.functions:
        for blk in f.blocks:
            insts = list(blk.instructions)
            out = []
            changed = False
            for inst in insts:
                si = getattr(inst, "sync_info", None)
                waits = list(si.on_wait) if si is not None and si.on_wait else []
                tname = type(inst).__name__
                limit = 2 if tname == "InstEventSemaphore" else (
                    0 if tname == "InstISA" else 1
                )
                if len(waits) > limit:
                    keep, excess = waits[:limit], waits[limit:]
                    for k in range(0, len(excess), 2):
                        es = mybir.InstEventSemaphore(
                            name=f"I-waitfix-{nfix}", ins=[], outs=[]
                        )
                        nfix += 1
                        es.engine = inst.engine
                        es.sync_info = mybir.SyncInfo(
                            on_wait=list(excess[k : k + 2]), on_update=[]
                        )
                        nc.register_instruction(es)
                        out.append(es)
                    inst.sync_info = mybir.SyncInfo(
                        on_wait=keep, on_update=list(si.on_update or [])
                    )
                    changed = True
                out.append(inst)
            if changed:
                blk.instructions = out
    return nfix


# ---------------------------------------------------------------------------
# Host-side input prep
# ---------------------------------------------------------------------------

def _rope_cache_np(seq, d):
    inv_freq = 1.0 / (ROPE_BASE ** (np.arange(0, d, 2, dtype=np.float32) / d))
    pos = np.arange(seq, dtype=np.float32)
    rot = pos[:, None] * inv_freq[None, :].astype(np.float32)
    theta = np.concatenate([rot, rot], axis=-1)  # [s, d]
    return np.cos(theta).astype(np.float32), np.sin(theta).astype(np.float32)


def _swizzle_kc(a2d):
    """[n_kc*128, F] -> [128, n_kc*F] (partition-contiguous SBUF layout)."""
    n_kc = a2d.shape[0] // 128
    return np.ascontiguousarray(
        a2d.reshape(n_kc, 128, a2d.shape[1]).transpose(1, 0, 2).reshape(128, -1)
    )


def _swizzle_x(x2d):
    """[hid, seq] -> [128, n*kc*512] with seq-chunk-major (n, kc) order."""
    n_kc = x2d.shape[0] // 128
    n_sc = x2d.shape[1] // SQ
    a = x2d.reshape(n_kc, 128, n_sc, SQ).transpose(1, 2, 0, 3)
    return np.ascontiguousarray(a.reshape(128, -1))


def make_in_maps(hidden_states, Wq, Wk, Wv, Wo):
    import ml_dtypes

    bf = ml_dtypes.bfloat16
    cos, sin = _rope_cache_np(SEQ, D)
    cosT = np.ascontiguousarray(cos.T).astype(bf)
    sinSv = sin.T.copy()
    sinSv[0:64, :] *= -1.0  # rotate_half sign, folded into the sin table
    sinSv = np.ascontiguousarray(sinSv).astype(bf)
    tri = (np.arange(KB)[:, None] <= np.arange(KB)[None, :]).astype(np.float32)
    triS = tri.astype(bf)
    ones = np.ones((128, 128), dtype=bf)

    in_maps = []
    for core in range(N_CORES):
        b = core // N_HGROUPS
        g = core % N_HGROUPS
        rs = slice(g * H_LOCAL * D, (g + 1) * H_LOCAL * D)
        in_maps.append(
            {
                "xS": _swizzle_x(hidden_states[b].T).astype(bf),
                "wqS": _swizzle_kc(Wq[rs, :].T).astype(bf),
                "wkS": _swizzle_kc(Wk[rs, :].T).astype(bf),
                "wvS": _swizzle_kc(Wv[rs, :].T).astype(bf),
                "woS": _swizzle_kc(Wo[:, rs].T).astype(bf),
                "cosT": cosT,
                "sinS": sinSv,
                "triS": triS,
                "ones": ones,
            }
        )
    return in_maps


def combine_outputs(results):
    """results: list of 8 dicts with 'out' [SEQ, HIDDEN] bf16 -> fp32 full."""
    out = np.zeros((BATCH, SEQ, HIDDEN), dtype=np.float32)
    for core, r in enumerate(results):
        b = core // N_HGROUPS
        out[b] += np.asarray(r["out"], dtype=np.float32)
    return out


_CACHE = {}


def run_hw(inputs, trace=False, **kw):
    """Run on 8 NeuronCores; returns (output, BassKernelResults)."""
    from concourse.bass_utils import run_bass_kernel_spmd

    if "nc" not in _CACHE:
        _CACHE["nc"] = build_bass()
    nc = _CACHE["nc"]
    in_maps = make_in_maps(
        np.asarray(inputs["hidden_states"], dtype=np.float32),
        np.asarray(inputs["Wq"], dtype=np.float32),
        np.asarray(inputs["Wk"], dtype=np.float32),
        np.asarray(inputs["Wv"], dtype=np.float32),
        np.asarray(inputs["Wo"], dtype=np.float32),
    )
    res = run_bass_kernel_spmd(
        nc, in_maps, core_ids=list(range(N_CORES)), trace=trace, **kw
    )
    return combine_outputs(res.results), res


def kernel(hidden_states, Wq, Wk, Wv, Wo):
    out, _ = run_hw(
        {
            "hidden_states": hidden_states,
            "Wq": Wq,
            "Wk": Wk,
            "Wv": Wv,
            "Wo": Wo,
        }
    )
    return out
